# revision 30
# baseline (speedup 1.0000x reference)
"""nn_HR2O_NL on 8 trn2 NeuronCores via Bass/Tile.

Pipeline (matches the oracle):
  q,k,v = conv3x3(x, w_{q,k,v});  att = softmax_j(q.k/sqrt(C)) per pixel;
  virt = att @ v;  GroupNorm(1,C) + affine + relu;  conv3x3(w_o);  out = x + .

Distribution:
  Phase 1 (rows-sharded): core c computes q,k,v + attention for image rows
  [4c, 4c+4) of all 32 instances (attention is independent per pixel and
  needs all instances at that pixel). GroupNorm partial stats (sum, sumsq
  per instance) are computed here too.
  Four AllToAlls (one per row, so comm pipelines against attention compute)
  reshard virt from row-shards to instance-shards. The last one also
  carries the fp32 GN partial stats (bitcast into the bf16 payload);
  every core then sums the 8 partials for exactly its own 4 instances.
  Phase 2: GroupNorm + affine + relu + conv3x3(w_o) fully local.
  Residual add happens on host.

Shapes hardcoded: x [32,512,32,32] f32, w_* [512,512,3,3] f32,
gamma/beta [512] f32. Output [32,512,32,32] f32.
"""
import os
import numpy as np
import ml_dtypes

N_CORES = 8
N, C, H, W = 32, 512, 32, 32
RPC = H // N_CORES          # rows per core = 4
IPC = N // N_CORES          # instances per core = 4
CB = C // 128               # channel blocks = 4
MB_QKV = 3 * CB             # 12 m-blocks for stacked q/k/v
KT = 9 * CB                 # 36 k-tiles (cb, tap) cb-major
PIX = RPC * W               # 128 pixels per core in phase 1
QPIX = W                    # pixels per quarter (one row)
PAY = C * IPC * W           # a2a payload bf16 elements per chunk = 65536
TAIL = IPC * 2 * 2          # stats tail: 4 inst x 2 stats x fp32-as-2xbf16
EPS = 1e-5
_STATE = {}


def _build():
    import concourse.bass as bass
    import concourse.tile as tile
    from concourse import bacc, mybir
    from concourse.masks import make_identity

    bf16 = mybir.dt.bfloat16
    f32 = mybir.dt.float32
    AF = mybir.ActivationFunctionType
    ALU = mybir.AluOpType
    AX = mybir.AxisListType

    nc = bacc.Bacc("TRN2", target_bir_lowering=False, debug=False,
                   num_devices=N_CORES)

    # ---- I/O ----
    xin_d = nc.dram_tensor("xin", [CB, 128, N, RPC + 2, W + 2], bf16,
                           kind="ExternalInput").ap()
    wqkv_d = nc.dram_tensor("wqkv", [MB_QKV, 128, KT, 128], bf16,
                            kind="ExternalInput").ap()
    wo_d = nc.dram_tensor("wo", [CB, 128, KT, 128], bf16,
                          kind="ExternalInput").ap()
    gb_d = nc.dram_tensor("gb", [128, 2 * CB], f32, kind="ExternalInput").ap()
    out_d = nc.dram_tensor("out", [IPC, C, H, W], bf16,
                           kind="ExternalOutput").ap()

    inv_cnt = 1.0 / float(C * H * W)
    att_scale = 1.0 / float(np.sqrt(C))

    with tile.TileContext(nc) as tc:
        with tc.tile_pool(name="const", bufs=1) as const, \
             tc.tile_pool(name="dram", bufs=1, space="DRAM") as dram, \
             tc.tile_pool(name="persist", bufs=1) as persist:

            ident = const.tile([128, 128], bf16)
            make_identity(nc, ident[:])
            ones128 = const.tile([128, 1], f32)
            nc.any.memset(ones128[:], 1.0)
            ones1 = const.tile([1, 128], f32)
            nc.any.memset(ones1[:], 1.0)
            gb_sb = const.tile([128, 2 * CB], f32)
            nc.sync.dma_start(gb_sb[:], gb_d[:])
            eps1 = const.tile([1, 1], f32)
            nc.any.memset(eps1[:], EPS)

            a2a_in = [dram.tile([N_CORES, PAY], bf16, name=f"a2ai{q}",
                                tag=f"a2ai{q}") for q in range(RPC)]
            a2a_out = [dram.tile([N_CORES, PAY], bf16, name=f"a2ao{q}",
                                 tag=f"a2ao{q}") for q in range(RPC)]
            ast_in = dram.tile([N_CORES, TAIL], bf16)
            ast_out = dram.tile([N_CORES, TAIL], bf16)

            # qkv activations: [128, mb(12), pix(128), n(32)] bf16
            qkv = persist.tile([128, MB_QKV, PIX, N], bf16)
            # GN partial stats: cols (kind2, cb4, n32, q4) fp32
            stats = persist.tile([128, 2, CB, N, RPC], f32)

            # ---------------- Phase 1a: conv3x3 -> q,k,v ----------------
            with tc.tile_pool(name="xin_pool", bufs=1) as xin_pool, \
                 tc.tile_pool(name="wq_pool", bufs=2) as wq_pool, \
                 tc.tile_pool(name="ps_conv", bufs=1, space="PSUM") as ps_conv:

                xin = xin_pool.tile([128, CB, N, RPC + 2, W + 2], bf16)
                for cb in range(CB):
                    nc.sync.dma_start(xin[:, cb], xin_d[cb])

                for mb in range(MB_QKV):
                    wt = wq_pool.tile([128, KT, 128], bf16, name=f"wt{mb}",
                                      tag="wt", bufs=2)
                    nc.sync.dma_start(wt[:], wqkv_d[mb])
                    pts = [ps_conv.tile([128, 16, W], f32, name=f"cpt{mb}_{ch}",
                                        tag=f"cpt{ch}") for ch in range(8)]
                    for kt in range(KT):
                        cb, tap = divmod(kt, 9)
                        dy, dx = divmod(tap, 3)
                        for ch in range(8):
                            yl, nh = divmod(ch, 2)
                            rhs = xin[:, cb, nh * 16:(nh + 1) * 16,
                                      yl + dy, dx:dx + W]
                            nc.tensor.matmul(pts[ch][:], wt[:, kt], rhs,
                                             start=(kt == 0),
                                             stop=(kt == KT - 1))
                    for ch in range(8):
                        yl, nh = divmod(ch, 2)
                        # psum [128, 16n, 32w] -> qkv[:, mb, yl*32+w, nh*16+n]
                        dst = qkv[:, mb, yl * W:(yl + 1) * W,
                                  nh * 16:(nh + 1) * 16]
                        src = pts[ch][:].rearrange("p n w -> p w n")
                        nc.any.tensor_copy(dst, src)

            # ---------------- Phase 1b: per-pixel attention ----------------
            with tc.tile_pool(name="att_small", bufs=2) as att_small, \
                 tc.tile_pool(name="virt_pool", bufs=1) as virt_pool, \
                 tc.tile_pool(name="ps_att", bufs=1, space="PSUM") as ps_att:
                for q in range(RPC):
                    # n-major so the a2a DMA source is coarse-run
                    virtT = virt_pool.tile([128, CB, N, QPIX], bf16,
                                           name=f"virtT{q}", tag="virtT",
                                           bufs=2)
                    for g in range(2):
                        grp = q * 2 + g
                        p0 = grp * 16
                        att_ps = ps_att.tile([32, 16, 32], f32,
                                             name=f"attps{grp}", tag="attps",
                                             bufs=2)
                        for pm in range(16):
                            pix = p0 + pm
                            for cb in range(CB):
                                nc.tensor.matmul(
                                    att_ps[:, pm],
                                    qkv[:, cb, pix, :],        # q lhsT [128,32]
                                    qkv[:, CB + cb, pix, :],   # k rhs  [128,32]
                                    start=(cb == 0), stop=(cb == CB - 1))
                        ex = att_small.tile([32, 16, 32], f32, name=f"ex{grp}",
                                            tag="ex", bufs=2)
                        nc.scalar.activation(ex[:], att_ps[:], AF.Exp,
                                             scale=att_scale)
                        sums = att_small.tile([32, 16], f32, name=f"sums{grp}",
                                              tag="sums", bufs=2)
                        nc.vector.tensor_reduce(sums[:], ex[:], axis=AX.X,
                                                op=ALU.add)
                        rec = att_small.tile([32, 16], f32, name=f"rec{grp}",
                                             tag="rec", bufs=2)
                        nc.vector.reciprocal(rec[:], sums[:])
                        attS = att_small.tile([32, 16, 32], bf16,
                                              name=f"attS{grp}", tag="attS",
                                              bufs=2)
                        for pm in range(16):
                            nc.vector.tensor_scalar(attS[:, pm], ex[:, pm],
                                                    rec[:, pm:pm + 1], None,
                                                    op0=ALU.mult)
                        attT = att_small.tile([32, 16, 32], bf16,
                                              name=f"attT{grp}", tag="attT",
                                              bufs=2)
                        nc.vector.transpose(attT[:], attS[:])

                        virt_ps = [ps_att.tile([128, 16, 32], f32,
                                               name=f"vps{grp}_{cb}",
                                               tag=f"vps{cb}")
                                   for cb in range(CB)]
                        for pm in range(16):
                            pix = p0 + pm
                            vT_ps = ps_att.tile([32, CB, 128], bf16,
                                                name=f"vTps{grp}_{pm}",
                                                tag="vTps", bufs=2)
                            for cb in range(CB):
                                nc.tensor.transpose(vT_ps[:, cb],
                                                    qkv[:, 2 * CB + cb, pix, :],
                                                    ident[:])
                            vT_sb = att_small.tile([32, CB, 128], bf16,
                                                   name=f"vTsb{grp}_{pm}",
                                                   tag="vTsb", bufs=3)
                            nc.any.tensor_copy(vT_sb[:], vT_ps[:])
                            for cb in range(CB):
                                nc.tensor.matmul(virt_ps[cb][:, pm],
                                                 vT_sb[:, cb],
                                                 attT[:, pm],
                                                 start=True, stop=True)
                        for cb in range(CB):
                            dst = virtT[:, cb, :, g * 16:(g + 1) * 16]
                            src = virt_ps[cb][:].rearrange("p x n -> p n x")
                            nc.any.tensor_copy(dst, src)

                    # GN partial stats for this quarter: per (cb, n)
                    sq = att_small.tile([128, N * QPIX], bf16,
                                        name=f"sq{q}", tag="sq", bufs=2)
                    for cb in range(CB):
                        nc.vector.tensor_reduce(stats[:, 0, cb, :, q],
                                                virtT[:, cb], axis=AX.X,
                                                op=ALU.add)
                        nc.scalar.activation(
                            sq.rearrange("p (n x) -> p n x", n=N)[:],
                            virtT[:, cb], AF.Square)
                        nc.vector.tensor_reduce(
                            stats[:, 1, cb, :, q],
                            sq.rearrange("p (n x) -> p n x", n=N)[:],
                            axis=AX.X, op=ALU.add)

                    # ship this quarter to its AllToAll bounce buffer;
                    # chunk layout (c, n, pix): one fat DMA per cb
                    for cb in range(CB):
                        dst = a2a_in[q][:, cb * 128 * IPC * QPIX:
                                        (cb + 1) * 128 * IPC * QPIX]
                        dst = dst.rearrange("d (c x) -> c d x", c=128)
                        src = virtT[:, cb].rearrange(
                            "p (d i) x -> p d (i x)", d=N_CORES)
                        eng = [nc.sync, nc.scalar, nc.sync, nc.scalar][cb]
                        eng.dma_start(dst, src)
                    if q < 3:
                        nc.gpsimd.collective_compute(
                            "AllToAll", ALU.bypass,
                            replica_groups=[list(range(N_CORES))],
                            ins=[a2a_in[q][:].opt()],
                            outs=[a2a_out[q][:].opt()])

            # local stat combine + fp32-bitcast tail, then last AllToAll
            with tc.tile_pool(name="stt", bufs=1) as stt, \
                 tc.tile_pool(name="ps_st1", bufs=1, space="PSUM") as ps_st1:
                sredq = stt.tile([128, 2, CB, N], f32)
                nc.vector.tensor_reduce(sredq[:], stats[:],
                                        axis=AX.X, op=ALU.add)
                stat_ps = ps_st1.tile([1, 2 * CB * N], f32)
                nc.tensor.matmul(
                    stat_ps[:], ones128[:],
                    sredq[:].rearrange("p a c n -> p (a c n)"),
                    start=True, stop=True)
                statv = stt.tile([1, 2 * CB * N], f32)
                nc.any.tensor_copy(statv[:], stat_ps[:])
                # reduce over cb -> statf [1, n, k] (n-major for a2a chunks)
                statf = stt.tile([1, N, 2], f32)
                nc.vector.tensor_reduce(
                    statf[:].rearrange("p n k -> p k n"),
                    statv[:].rearrange("p (k c n) -> p k n c", k=2, c=CB),
                    axis=AX.X, op=ALU.add)
                for d in range(N_CORES):
                    nc.sync.dma_start(
                        ast_in[d],
                        statf[:, 4 * d:4 * d + 4, :].bitcast(bf16))
                # tiny stats AllToAll first: its result lands while the
                # last payload AllToAll is still transferring
                nc.gpsimd.collective_compute(
                    "AllToAll", ALU.bypass,
                    replica_groups=[list(range(N_CORES))],
                    ins=[ast_in[:].opt()], outs=[ast_out[:].opt()])
                nc.gpsimd.collective_compute(
                    "AllToAll", ALU.bypass,
                    replica_groups=[list(range(N_CORES))],
                    ins=[a2a_in[3][:].opt()],
                    outs=[a2a_out[3][:].opt()])

            # ---------------- Phase 2: GN + relu + conv_o ----------------
            with tc.tile_pool(name="p2", bufs=1) as p2:
                # sum the 8 cores' stat partials for my 4 instances
                statAb = p2.tile([1, N_CORES, TAIL], bf16)
                nc.sync.dma_start(statAb[:], ast_out[:])
                statL = p2.tile([1, IPC, 2], f32)
                nc.vector.tensor_reduce(
                    statL[:].rearrange("p n k -> p (n k)"),
                    statAb[:].bitcast(f32).rearrange("p s x -> p x s"),
                    axis=AX.X, op=ALU.add)
                # var*cnt^2 = S2*cnt - S1^2 ; rstd = 1/sqrt(var + eps)
                msq = p2.tile([1, IPC], f32)
                nc.vector.tensor_mul(msq[:], statL[:, :, 0], statL[:, :, 0])
                varr = p2.tile([1, IPC], f32)
                nc.vector.scalar_tensor_tensor(
                    varr[:], statL[:, :, 1], 1.0 / inv_cnt, msq[:],
                    op0=ALU.mult, op1=ALU.subtract)
                std = p2.tile([1, IPC], f32)
                nc.scalar.activation(std[:], varr[:], AF.Sqrt, bias=eps1[:],
                                     scale=inv_cnt * inv_cnt)
                rn8 = p2.tile([1, 2 * IPC], f32)
                nc.vector.reciprocal(rn8[:, 0:IPC], std[:])
                nmr = p2.tile([1, IPC], f32)
                nc.vector.tensor_mul(nmr[:], statL[:, :, 0], rn8[:, 0:IPC])
                nc.vector.tensor_scalar(rn8[:, IPC:2 * IPC], nmr[:],
                                        -inv_cnt, None, op0=ALU.mult)
                with tc.tile_pool(name="ps_st2", bufs=1,
                                  space="PSUM") as ps_st2:
                    bc_ps = ps_st2.tile([128, 2 * IPC], f32)
                    nc.tensor.matmul(bc_ps[:], ones1[:], rn8[:],
                                     start=True, stop=True)
                    bc = p2.tile([128, 2 * IPC], f32)
                    nc.any.tensor_copy(bc[:], bc_ps[:])
                # A = rstd*gamma_c per (cb, n); B = beta_c - mean*rstd*gamma_c
                A = p2.tile([128, CB, IPC], f32)
                B = p2.tile([128, CB, IPC], f32)
                for cb in range(CB):
                    nc.vector.tensor_scalar(A[:, cb], bc[:, 0:IPC],
                                            gb_sb[:, cb:cb + 1], None,
                                            op0=ALU.mult)
                    nc.vector.tensor_scalar(B[:, cb], bc[:, IPC:2 * IPC],
                                            gb_sb[:, cb:cb + 1], None,
                                            op0=ALU.mult)
                    nc.vector.tensor_scalar(B[:, cb], B[:, cb],
                                            gb_sb[:, CB + cb:CB + cb + 1],
                                            None, op0=ALU.add)

                # assemble padded conv input [128, cb, n, 34, 34]
                convo = p2.tile([128, CB, IPC, H + 2, W + 2], bf16)
                nc.any.memset(convo[:], 0.0)
                for q in range(RPC):
                    staged = p2.tile([128, CB, N_CORES, IPC, W],
                                     bf16, name=f"staged{q}",
                                     tag=f"staged{q}")
                    for cb in range(CB):
                        src = a2a_out[q][:, cb * 128 * IPC * W:
                                         (cb + 1) * 128 * IPC * W]
                        src = src.rearrange("s (c x) -> c s x", c=128)
                        dst = staged[:, cb].rearrange("p s i x -> p s (i x)")
                        eng = [nc.sync, nc.scalar, nc.sync, nc.scalar][cb]
                        eng.dma_start(dst, src)
                    for cb in range(CB):
                        # global rows y = 4*src + q (pad offset +1); all n
                        t = convo[:, cb, :, 1:H + 1, :]
                        t = t.rearrange("p n (s y) w -> p n s y w", y=4)
                        dst = t[:, :, :, q, 1:W + 1]
                        src = staged[:, cb].rearrange("p s n x -> p n s x")
                        nc.vector.tensor_copy(dst, src)
                # GN affine + relu on interior only (padding must stay 0)
                for cb in range(CB):
                    for n in range(IPC):
                        ap = convo[:, cb, n, 1:H + 1, 1:W + 1]
                        nc.scalar.activation(ap, ap, AF.Relu,
                                             scale=A[:, cb, n:n + 1],
                                             bias=B[:, cb, n:n + 1])

                with tc.tile_pool(name="wo_pool", bufs=2) as wo_pool, \
                     tc.tile_pool(name="out_pool", bufs=4) as out_pool, \
                     tc.tile_pool(name="ps_co", bufs=1, space="PSUM") as ps_co:
                    for mb in range(CB):
                        wt2 = wo_pool.tile([128, KT, 128], bf16,
                                           name=f"wo{mb}", tag="wo", bufs=2)
                        nc.sync.dma_start(wt2[:], wo_d[mb])
                        pts2 = [ps_co.tile([128, 16, W], f32,
                                           name=f"opt{mb}_{ch}",
                                           tag=f"opt{ch}") for ch in range(8)]
                        for kt in range(KT):
                            cb, tap = divmod(kt, 9)
                            dy, dx = divmod(tap, 3)
                            for ch in range(8):
                                n, yh = divmod(ch, 2)
                                rhs = convo[:, cb, n,
                                            yh * 16 + dy:yh * 16 + dy + 16,
                                            dx:dx + W]
                                nc.tensor.matmul(pts2[ch][:], wt2[:, kt], rhs,
                                                 start=(kt == 0),
                                                 stop=(kt == KT - 1))
                        for ch in range(8):
                            n, yh = divmod(ch, 2)
                            ob = out_pool.tile([128, 16, W], bf16,
                                               name=f"ob{mb}_{ch}", tag="ob",
                                               bufs=4)
                            nc.any.tensor_copy(ob[:], pts2[ch][:])
                            nc.sync.dma_start(
                                out_d[n, mb * 128:(mb + 1) * 128,
                                      yh * 16:(yh + 1) * 16, :],
                                ob[:])

    _compile_with_ldw_dedupe(nc, mybir)
    return nc


def _dedupe_ldweights(nc, mybir):
    """Drop Ldweights whose weights AP is identical to the PE array's
    current contents (consecutive matmuls sharing the same lhsT). The PE
    sequencer dispatches ~1 instruction per ~130ns, so at N=512 the
    per-matmul Ldweights makes the pair dispatch-bound; the conv loops
    reuse each lhsT for 8 consecutive matmuls, so 7/8 are redundant."""
    removed = 0
    for blk in nc.main_func.blocks:
        keep = []
        last_sig = None
        for inst in blk.instructions:
            if isinstance(inst, mybir.InstLdweights):
                si = inst.sync_info
                clean = si is None or (not si.on_wait and not si.on_update)
                sig = (str(inst.ins[0]), str(getattr(inst, "tile_size", None)),
                       str(getattr(inst, "tile_position", None)))
                if clean and sig == last_sig:
                    removed += 1
                    continue
                last_sig = sig
            elif isinstance(inst, mybir.InstMatmult):
                if getattr(inst, "ldweights", False):
                    last_sig = None
            keep.append(inst)
        blk.instructions[:] = keep
    return removed


def _compile_with_ldw_dedupe(nc, mybir):
    # bacc.Bacc.compile() with a dedupe pass after matmul waits move to
    # the Ldweights instructions (so waits we must keep stay visible).
    from concourse import inst_simplify

    nc.insert_bir_kernel_barrier_sem_inc()
    nc.move_matmul_waits_to_ldweights()
    _dedupe_ldweights(nc, mybir)
    nc.generate_event_semaphores()
    nc.remove_dead_instructions_after_branch()
    nc.validate_blocks()
    nc.dce_regs()
    nc.thread_jumps()
    nc.remove_dead_blocks()
    nc.remove_dead_allocations()
    nc.verify_switch_hints()
    nc.alloc_regs()
    inst_simplify.simplify(nc)
    nc.fuse_regops()
    nc.fuse_blocks()
    nc.replace_nops_with_events()
    for engine in nc.engines:
        nc.fuse_nops(engine)
    nc.remove_dead_nops()
    nc.remove_dangling_data()
    nc.generate_event_semaphores()
    nc.insert_library_loads()
    nc.insert_act_table_loads()
    nc.insert_hostgen_rebases()
    nc.codegen_inst_isa_subclasses()


def _prep_inputs(x, w_q, w_k, w_v, w_o, gamma, beta):
    bf = ml_dtypes.bfloat16
    x = np.asarray(x, np.float32)
    # x -> [c, n, y_padded(36), w_padded(34)] bf16 (halo rows + conv pad rows
    # share the same zero-fill)
    xt = np.ascontiguousarray(x.transpose(1, 0, 2, 3))          # [512,32,32,32]
    xpad = np.zeros((C, N, H + 4, W + 2), dtype=bf)
    xpad[:, :, 2:H + 2, 1:W + 1] = xt
    xins = []
    for c8 in range(N_CORES):
        sl = xpad[:, :, 4 * c8 + 1:4 * c8 + 7, :]               # [512,32,6,34]
        xins.append(np.ascontiguousarray(sl).reshape(CB, 128, N, RPC + 2, W + 2))

    def prep_w(*ws):
        Wf = np.concatenate([np.asarray(w, np.float32).reshape(C, C, 9)
                             for w in ws], axis=0)              # [M, 512, 9]
        M = Wf.shape[0]
        # dest [mb, c_part, kt=(cb,tap), m]; src [mb*128+m, cb*128+c, tap]
        Wr = Wf.reshape(M // 128, 128, CB, 128, 9)              # [mb,m,cb,c,tap]
        Wt = Wr.transpose(0, 3, 2, 4, 1)                        # [mb,c,cb,tap,m]
        return np.ascontiguousarray(Wt).astype(bf).reshape(M // 128, 128, KT, 128)

    wqkv = prep_w(w_q, w_k, w_v)
    wo = prep_w(w_o)
    gb = np.empty((128, 2 * CB), np.float32)
    gb[:, 0:CB] = np.asarray(gamma, np.float32).reshape(CB, 128).T
    gb[:, CB:2 * CB] = np.asarray(beta, np.float32).reshape(CB, 128).T
    return xins, wqkv, wo, gb


def kernel(x, w_q, w_k, w_v, w_o, gamma, beta):
    from concourse.bass_utils import run_bass_kernel_spmd

    if "nc" not in _STATE:
        _STATE["nc"] = _build()
    nc = _STATE["nc"]

    x = np.asarray(x, np.float32)
    xins, wqkv, wo, gb = _prep_inputs(x, w_q, w_k, w_v, w_o, gamma, beta)
    in_maps = [{"xin": xins[c], "wqkv": wqkv, "wo": wo, "gb": gb}
               for c in range(N_CORES)]

    res = run_bass_kernel_spmd(nc, in_maps, core_ids=list(range(N_CORES)),
                               tmpdir=os.environ.get("HR2O_TMPDIR"))
    _STATE["last_results"] = res

    out = np.array(x, copy=True)
    for c in range(N_CORES):
        virt = res.results[c]["out"].astype(np.float32)   # [4, 512, 32, 32]
        out[4 * c:4 * c + 4] += virt
    return out


# revision 31
# speedup vs baseline: 1.0708x; 1.0708x over previous
"""nn_HR2O_NL on 8 trn2 NeuronCores via Bass/Tile.

Pipeline (matches the oracle):
  q,k,v = conv3x3(x, w_{q,k,v});  att = softmax_j(q.k/sqrt(C)) per pixel;
  virt = att @ v;  GroupNorm(1,C) + affine + relu;  conv3x3(w_o);  out = x + .

Distribution:
  Phase 1 (rows-sharded): core c computes q,k,v + attention for image rows
  [4c, 4c+4) of all 32 instances (attention is independent per pixel and
  needs all instances at that pixel). GroupNorm partial stats (sum, sumsq
  per instance) are computed here too.
  Four AllToAlls (one per row, so comm pipelines against attention compute)
  reshard virt from row-shards to instance-shards. The last one also
  carries the fp32 GN partial stats (bitcast into the bf16 payload);
  every core then sums the 8 partials for exactly its own 4 instances.
  Phase 2: GroupNorm + affine + relu + conv3x3(w_o) fully local.
  Residual add happens on host.

Shapes hardcoded: x [32,512,32,32] f32, w_* [512,512,3,3] f32,
gamma/beta [512] f32. Output [32,512,32,32] f32.
"""
import os
import numpy as np
import ml_dtypes

N_CORES = 8
N, C, H, W = 32, 512, 32, 32
RPC = H // N_CORES          # rows per core = 4
IPC = N // N_CORES          # instances per core = 4
CB = C // 128               # channel blocks = 4
MB_QKV = 3 * CB             # 12 m-blocks for stacked q/k/v
KT = 9 * CB                 # 36 k-tiles (cb, tap) cb-major
PIX = RPC * W               # 128 pixels per core in phase 1
QPIX = W                    # pixels per quarter (one row)
PAY = C * IPC * W           # a2a payload bf16 elements per chunk = 65536
TAIL = IPC * 2 * 2          # stats tail: 4 inst x 2 stats x fp32-as-2xbf16
EPS = 1e-5
_STATE = {}


def _build():
    import concourse.bass as bass
    import concourse.tile as tile
    from concourse import bacc, mybir
    from concourse.masks import make_identity

    bf16 = mybir.dt.bfloat16
    f32 = mybir.dt.float32
    AF = mybir.ActivationFunctionType
    ALU = mybir.AluOpType
    AX = mybir.AxisListType

    nc = bacc.Bacc("TRN2", target_bir_lowering=False, debug=False,
                   num_devices=N_CORES)

    # ---- I/O ----
    xin_d = nc.dram_tensor("xin", [CB, 128, N, RPC + 2, W + 2], bf16,
                           kind="ExternalInput").ap()
    wqkv_d = nc.dram_tensor("wqkv", [MB_QKV, 128, KT, 128], bf16,
                            kind="ExternalInput").ap()
    wo_d = nc.dram_tensor("wo", [CB, 128, KT, 128], bf16,
                          kind="ExternalInput").ap()
    gb_d = nc.dram_tensor("gb", [128, 2 * CB], f32, kind="ExternalInput").ap()
    out_d = nc.dram_tensor("out", [IPC, C, H, W], bf16,
                           kind="ExternalOutput").ap()

    inv_cnt = 1.0 / float(C * H * W)
    att_scale = 1.0 / float(np.sqrt(C))

    with tile.TileContext(nc) as tc:
        with tc.tile_pool(name="const", bufs=1) as const, \
             tc.tile_pool(name="dram", bufs=1, space="DRAM") as dram, \
             tc.tile_pool(name="persist", bufs=1) as persist:

            ident = const.tile([128, 128], bf16)
            make_identity(nc, ident[:])
            ones128 = const.tile([128, 1], f32)
            nc.any.memset(ones128[:], 1.0)
            ones1 = const.tile([1, 128], f32)
            nc.any.memset(ones1[:], 1.0)
            gb_sb = const.tile([128, 2 * CB], f32)
            nc.sync.dma_start(gb_sb[:], gb_d[:])
            eps1 = const.tile([1, 1], f32)
            nc.any.memset(eps1[:], EPS)

            # quarter q < 3: pure payload; quarter 3 carries stats tail
            a2a_in = [dram.tile([N_CORES, PAY + (TAIL if q == 3 else 0)],
                                bf16, name=f"a2ai{q}", tag=f"a2ai{q}")
                      for q in range(RPC)]
            a2a_out = [dram.tile([N_CORES, PAY + (TAIL if q == 3 else 0)],
                                 bf16, name=f"a2ao{q}", tag=f"a2ao{q}")
                       for q in range(RPC)]

            # qkv activations: [128, mb(12), pix(128), n(32)] bf16
            qkv = persist.tile([128, MB_QKV, PIX, N], bf16)
            # GN partial stats: cols (kind2, cb4, n32, q4) fp32
            stats = persist.tile([128, 2, CB, N, RPC], f32)

            # ---------------- Phase 1a: conv3x3 -> q,k,v ----------------
            with tc.tile_pool(name="xin_pool", bufs=1) as xin_pool, \
                 tc.tile_pool(name="wq_pool", bufs=2) as wq_pool, \
                 tc.tile_pool(name="ps_conv", bufs=1, space="PSUM") as ps_conv:

                xin = xin_pool.tile([128, CB, N, RPC + 2, W + 2], bf16)
                for cb in range(CB):
                    nc.sync.dma_start(xin[:, cb], xin_d[cb])

                for mb in range(MB_QKV):
                    wt = wq_pool.tile([128, KT, 128], bf16, name=f"wt{mb}",
                                      tag="wt", bufs=2)
                    nc.sync.dma_start(wt[:], wqkv_d[mb])
                    pts = [ps_conv.tile([128, 16, W], f32, name=f"cpt{mb}_{ch}",
                                        tag=f"cpt{ch}") for ch in range(8)]
                    for kt in range(KT):
                        cb, tap = divmod(kt, 9)
                        dy, dx = divmod(tap, 3)
                        for ch in range(8):
                            yl, nh = divmod(ch, 2)
                            rhs = xin[:, cb, nh * 16:(nh + 1) * 16,
                                      yl + dy, dx:dx + W]
                            nc.tensor.matmul(pts[ch][:], wt[:, kt], rhs,
                                             start=(kt == 0),
                                             stop=(kt == KT - 1))
                    for ch in range(8):
                        yl, nh = divmod(ch, 2)
                        # psum [128, 16n, 32w] -> qkv[:, mb, yl*32+w, nh*16+n]
                        dst = qkv[:, mb, yl * W:(yl + 1) * W,
                                  nh * 16:(nh + 1) * 16]
                        src = pts[ch][:].rearrange("p n w -> p w n")
                        nc.any.tensor_copy(dst, src)

            # ---------------- Phase 1b: per-pixel attention ----------------
            with tc.tile_pool(name="att_small", bufs=2) as att_small, \
                 tc.tile_pool(name="virt_pool", bufs=1) as virt_pool, \
                 tc.tile_pool(name="ps_att", bufs=1, space="PSUM") as ps_att:
                for q in range(RPC):
                    # n-major so the a2a DMA source is coarse-run
                    virtT = virt_pool.tile([128, CB, N, QPIX], bf16,
                                           name=f"virtT{q}", tag="virtT",
                                           bufs=2)
                    for g in range(2):
                        grp = q * 2 + g
                        p0 = grp * 16
                        att_ps = ps_att.tile([32, 16, 32], f32,
                                             name=f"attps{grp}", tag="attps",
                                             bufs=2)
                        for pm in range(16):
                            pix = p0 + pm
                            for cb in range(CB):
                                nc.tensor.matmul(
                                    att_ps[:, pm],
                                    qkv[:, cb, pix, :],        # q lhsT [128,32]
                                    qkv[:, CB + cb, pix, :],   # k rhs  [128,32]
                                    start=(cb == 0), stop=(cb == CB - 1))
                        ex = att_small.tile([32, 16, 32], f32, name=f"ex{grp}",
                                            tag="ex", bufs=2)
                        nc.scalar.activation(ex[:], att_ps[:], AF.Exp,
                                             scale=att_scale)
                        sums = att_small.tile([32, 16], f32, name=f"sums{grp}",
                                              tag="sums", bufs=2)
                        nc.vector.tensor_reduce(sums[:], ex[:], axis=AX.X,
                                                op=ALU.add)
                        rec = att_small.tile([32, 16], f32, name=f"rec{grp}",
                                             tag="rec", bufs=2)
                        nc.vector.reciprocal(rec[:], sums[:])
                        attS = att_small.tile([32, 16, 32], bf16,
                                              name=f"attS{grp}", tag="attS",
                                              bufs=2)
                        for pm in range(16):
                            nc.vector.tensor_scalar(attS[:, pm], ex[:, pm],
                                                    rec[:, pm:pm + 1], None,
                                                    op0=ALU.mult)
                        attT = att_small.tile([32, 16, 32], bf16,
                                              name=f"attT{grp}", tag="attT",
                                              bufs=2)
                        nc.vector.transpose(attT[:], attS[:])

                        virt_ps = [ps_att.tile([128, 16, 32], f32,
                                               name=f"vps{grp}_{cb}",
                                               tag=f"vps{cb}")
                                   for cb in range(CB)]
                        for pm in range(16):
                            pix = p0 + pm
                            vT_ps = ps_att.tile([32, CB, 128], bf16,
                                                name=f"vTps{grp}_{pm}",
                                                tag="vTps", bufs=2)
                            for cb in range(CB):
                                nc.tensor.transpose(vT_ps[:, cb],
                                                    qkv[:, 2 * CB + cb, pix, :],
                                                    ident[:])
                            vT_sb = att_small.tile([32, CB, 128], bf16,
                                                   name=f"vTsb{grp}_{pm}",
                                                   tag="vTsb", bufs=3)
                            nc.any.tensor_copy(vT_sb[:], vT_ps[:])
                            for cb in range(CB):
                                nc.tensor.matmul(virt_ps[cb][:, pm],
                                                 vT_sb[:, cb],
                                                 attT[:, pm],
                                                 start=True, stop=True)
                        for cb in range(CB):
                            dst = virtT[:, cb, :, g * 16:(g + 1) * 16]
                            src = virt_ps[cb][:].rearrange("p x n -> p n x")
                            nc.any.tensor_copy(dst, src)

                    # GN partial stats for this quarter: per (cb, n)
                    sq = att_small.tile([128, N * QPIX], bf16,
                                        name=f"sq{q}", tag="sq", bufs=2)
                    for cb in range(CB):
                        nc.vector.tensor_reduce(stats[:, 0, cb, :, q],
                                                virtT[:, cb], axis=AX.X,
                                                op=ALU.add)
                        nc.scalar.activation(
                            sq.rearrange("p (n x) -> p n x", n=N)[:],
                            virtT[:, cb], AF.Square)
                        nc.vector.tensor_reduce(
                            stats[:, 1, cb, :, q],
                            sq.rearrange("p (n x) -> p n x", n=N)[:],
                            axis=AX.X, op=ALU.add)

                    # ship this quarter to its AllToAll bounce buffer;
                    # chunk layout (c, n, pix): one fat DMA per cb
                    for cb in range(CB):
                        dst = a2a_in[q][:, cb * 128 * IPC * QPIX:
                                        (cb + 1) * 128 * IPC * QPIX]
                        dst = dst.rearrange("d (c x) -> c d x", c=128)
                        src = virtT[:, cb].rearrange(
                            "p (d i) x -> p d (i x)", d=N_CORES)
                        eng = [nc.sync, nc.scalar, nc.sync, nc.scalar][cb]
                        eng.dma_start(dst, src)
                    if q < 3:
                        nc.gpsimd.collective_compute(
                            "AllToAll", ALU.bypass,
                            replica_groups=[list(range(N_CORES))],
                            ins=[a2a_in[q][:].opt()],
                            outs=[a2a_out[q][:].opt()])

            # local stat combine + fp32-bitcast tail, then last AllToAll
            with tc.tile_pool(name="stt", bufs=1) as stt, \
                 tc.tile_pool(name="ps_st1", bufs=1, space="PSUM") as ps_st1:
                sredq = stt.tile([128, 2, CB, N], f32)
                nc.vector.tensor_reduce(sredq[:], stats[:],
                                        axis=AX.X, op=ALU.add)
                stat_ps = ps_st1.tile([1, 2 * CB * N], f32)
                nc.tensor.matmul(
                    stat_ps[:], ones128[:],
                    sredq[:].rearrange("p a c n -> p (a c n)"),
                    start=True, stop=True)
                statv = stt.tile([1, 2 * CB * N], f32)
                nc.any.tensor_copy(statv[:], stat_ps[:])
                # reduce over cb -> statf [1, n, k] (n-major for a2a chunks)
                statf = stt.tile([1, N, 2], f32)
                nc.vector.tensor_reduce(
                    statf[:].rearrange("p n k -> p k n"),
                    statv[:].rearrange("p (k c n) -> p k n c", k=2, c=CB),
                    axis=AX.X, op=ALU.add)
                for d in range(N_CORES):
                    nc.sync.dma_start(
                        a2a_in[3][d, PAY:PAY + TAIL],
                        statf[:, 4 * d:4 * d + 4, :].bitcast(bf16))
                nc.gpsimd.collective_compute(
                    "AllToAll", ALU.bypass,
                    replica_groups=[list(range(N_CORES))],
                    ins=[a2a_in[3][:].opt()],
                    outs=[a2a_out[3][:].opt()])

            # ---------------- Phase 2: GN + relu + conv_o ----------------
            with tc.tile_pool(name="p2", bufs=1) as p2:
                # sum the 8 cores' stat partials for my 4 instances
                statAb = p2.tile([1, N_CORES, TAIL], bf16)
                nc.sync.dma_start(statAb[:], a2a_out[3][:, PAY:PAY + TAIL])
                statL = p2.tile([1, IPC, 2], f32)
                nc.vector.tensor_reduce(
                    statL[:].rearrange("p n k -> p (n k)"),
                    statAb[:].bitcast(f32).rearrange("p s x -> p x s"),
                    axis=AX.X, op=ALU.add)
                # var*cnt^2 = S2*cnt - S1^2 ; rstd = 1/sqrt(var + eps)
                msq = p2.tile([1, IPC], f32)
                nc.vector.tensor_mul(msq[:], statL[:, :, 0], statL[:, :, 0])
                varr = p2.tile([1, IPC], f32)
                nc.vector.scalar_tensor_tensor(
                    varr[:], statL[:, :, 1], 1.0 / inv_cnt, msq[:],
                    op0=ALU.mult, op1=ALU.subtract)
                std = p2.tile([1, IPC], f32)
                nc.scalar.activation(std[:], varr[:], AF.Sqrt, bias=eps1[:],
                                     scale=inv_cnt * inv_cnt)
                rn8 = p2.tile([1, 2 * IPC], f32)
                nc.vector.reciprocal(rn8[:, 0:IPC], std[:])
                nmr = p2.tile([1, IPC], f32)
                nc.vector.tensor_mul(nmr[:], statL[:, :, 0], rn8[:, 0:IPC])
                nc.vector.tensor_scalar(rn8[:, IPC:2 * IPC], nmr[:],
                                        -inv_cnt, None, op0=ALU.mult)
                with tc.tile_pool(name="ps_st2", bufs=1,
                                  space="PSUM") as ps_st2:
                    bc_ps = ps_st2.tile([128, 2 * IPC], f32)
                    nc.tensor.matmul(bc_ps[:], ones1[:], rn8[:],
                                     start=True, stop=True)
                    bc = p2.tile([128, 2 * IPC], f32)
                    nc.any.tensor_copy(bc[:], bc_ps[:])
                # A = rstd*gamma_c per (cb, n); B = beta_c - mean*rstd*gamma_c
                A = p2.tile([128, CB, IPC], f32)
                B = p2.tile([128, CB, IPC], f32)
                for cb in range(CB):
                    nc.vector.tensor_scalar(A[:, cb], bc[:, 0:IPC],
                                            gb_sb[:, cb:cb + 1], None,
                                            op0=ALU.mult)
                    nc.vector.tensor_scalar(B[:, cb], bc[:, IPC:2 * IPC],
                                            gb_sb[:, cb:cb + 1], None,
                                            op0=ALU.mult)
                    nc.vector.tensor_scalar(B[:, cb], B[:, cb],
                                            gb_sb[:, CB + cb:CB + cb + 1],
                                            None, op0=ALU.add)

                # assemble padded conv input [128, cb, n, 34, 34]
                convo = p2.tile([128, CB, IPC, H + 2, W + 2], bf16)
                nc.any.memset(convo[:], 0.0)
                for q in range(RPC):
                    staged = p2.tile([128, CB, N_CORES, IPC, W],
                                     bf16, name=f"staged{q}",
                                     tag=f"staged{q}")
                    for cb in range(CB):
                        src = a2a_out[q][:, cb * 128 * IPC * W:
                                         (cb + 1) * 128 * IPC * W]
                        src = src.rearrange("s (c x) -> c s x", c=128)
                        dst = staged[:, cb].rearrange("p s i x -> p s (i x)")
                        eng = [nc.sync, nc.scalar, nc.sync, nc.scalar][cb]
                        eng.dma_start(dst, src)
                    for cb in range(CB):
                        # global rows y = 4*src + q (pad offset +1); all n
                        t = convo[:, cb, :, 1:H + 1, :]
                        t = t.rearrange("p n (s y) w -> p n s y w", y=4)
                        dst = t[:, :, :, q, 1:W + 1]
                        src = staged[:, cb].rearrange("p s n x -> p n s x")
                        nc.vector.tensor_copy(dst, src)
                # GN affine + relu on interior only (padding must stay 0)
                for cb in range(CB):
                    for n in range(IPC):
                        ap = convo[:, cb, n, 1:H + 1, 1:W + 1]
                        nc.scalar.activation(ap, ap, AF.Relu,
                                             scale=A[:, cb, n:n + 1],
                                             bias=B[:, cb, n:n + 1])

                with tc.tile_pool(name="wo_pool", bufs=2) as wo_pool, \
                     tc.tile_pool(name="out_pool", bufs=4) as out_pool, \
                     tc.tile_pool(name="ps_co", bufs=1, space="PSUM") as ps_co:
                    for mb in range(CB):
                        wt2 = wo_pool.tile([128, KT, 128], bf16,
                                           name=f"wo{mb}", tag="wo", bufs=2)
                        nc.sync.dma_start(wt2[:], wo_d[mb])
                        pts2 = [ps_co.tile([128, 16, W], f32,
                                           name=f"opt{mb}_{ch}",
                                           tag=f"opt{ch}") for ch in range(8)]
                        for kt in range(KT):
                            cb, tap = divmod(kt, 9)
                            dy, dx = divmod(tap, 3)
                            for ch in range(8):
                                n, yh = divmod(ch, 2)
                                rhs = convo[:, cb, n,
                                            yh * 16 + dy:yh * 16 + dy + 16,
                                            dx:dx + W]
                                nc.tensor.matmul(pts2[ch][:], wt2[:, kt], rhs,
                                                 start=(kt == 0),
                                                 stop=(kt == KT - 1))
                        for ch in range(8):
                            n, yh = divmod(ch, 2)
                            ob = out_pool.tile([128, 16, W], bf16,
                                               name=f"ob{mb}_{ch}", tag="ob",
                                               bufs=4)
                            nc.any.tensor_copy(ob[:], pts2[ch][:])
                            nc.sync.dma_start(
                                out_d[n, mb * 128:(mb + 1) * 128,
                                      yh * 16:(yh + 1) * 16, :],
                                ob[:])

    _compile_with_ldw_dedupe(nc, mybir)
    return nc


def _dedupe_ldweights(nc, mybir):
    """Drop Ldweights whose weights AP is identical to the PE array's
    current contents (consecutive matmuls sharing the same lhsT). The PE
    sequencer dispatches ~1 instruction per ~130ns, so at N=512 the
    per-matmul Ldweights makes the pair dispatch-bound; the conv loops
    reuse each lhsT for 8 consecutive matmuls, so 7/8 are redundant."""
    removed = 0
    for blk in nc.main_func.blocks:
        keep = []
        last_sig = None
        for inst in blk.instructions:
            if isinstance(inst, mybir.InstLdweights):
                si = inst.sync_info
                clean = si is None or (not si.on_wait and not si.on_update)
                sig = (str(inst.ins[0]), str(getattr(inst, "tile_size", None)),
                       str(getattr(inst, "tile_position", None)))
                if clean and sig == last_sig:
                    removed += 1
                    continue
                last_sig = sig
            elif isinstance(inst, mybir.InstMatmult):
                if getattr(inst, "ldweights", False):
                    last_sig = None
            keep.append(inst)
        blk.instructions[:] = keep
    return removed


def _compile_with_ldw_dedupe(nc, mybir):
    # bacc.Bacc.compile() with a dedupe pass after matmul waits move to
    # the Ldweights instructions (so waits we must keep stay visible).
    from concourse import inst_simplify

    nc.insert_bir_kernel_barrier_sem_inc()
    nc.move_matmul_waits_to_ldweights()
    _dedupe_ldweights(nc, mybir)
    nc.generate_event_semaphores()
    nc.remove_dead_instructions_after_branch()
    nc.validate_blocks()
    nc.dce_regs()
    nc.thread_jumps()
    nc.remove_dead_blocks()
    nc.remove_dead_allocations()
    nc.verify_switch_hints()
    nc.alloc_regs()
    inst_simplify.simplify(nc)
    nc.fuse_regops()
    nc.fuse_blocks()
    nc.replace_nops_with_events()
    for engine in nc.engines:
        nc.fuse_nops(engine)
    nc.remove_dead_nops()
    nc.remove_dangling_data()
    nc.generate_event_semaphores()
    nc.insert_library_loads()
    nc.insert_act_table_loads()
    nc.insert_hostgen_rebases()
    nc.codegen_inst_isa_subclasses()


def _prep_inputs(x, w_q, w_k, w_v, w_o, gamma, beta):
    bf = ml_dtypes.bfloat16
    x = np.asarray(x, np.float32)
    # x -> [c, n, y_padded(36), w_padded(34)] bf16 (halo rows + conv pad rows
    # share the same zero-fill)
    xt = np.ascontiguousarray(x.transpose(1, 0, 2, 3))          # [512,32,32,32]
    xpad = np.zeros((C, N, H + 4, W + 2), dtype=bf)
    xpad[:, :, 2:H + 2, 1:W + 1] = xt
    xins = []
    for c8 in range(N_CORES):
        sl = xpad[:, :, 4 * c8 + 1:4 * c8 + 7, :]               # [512,32,6,34]
        xins.append(np.ascontiguousarray(sl).reshape(CB, 128, N, RPC + 2, W + 2))

    def prep_w(*ws):
        Wf = np.concatenate([np.asarray(w, np.float32).reshape(C, C, 9)
                             for w in ws], axis=0)              # [M, 512, 9]
        M = Wf.shape[0]
        # dest [mb, c_part, kt=(cb,tap), m]; src [mb*128+m, cb*128+c, tap]
        Wr = Wf.reshape(M // 128, 128, CB, 128, 9)              # [mb,m,cb,c,tap]
        Wt = Wr.transpose(0, 3, 2, 4, 1)                        # [mb,c,cb,tap,m]
        return np.ascontiguousarray(Wt).astype(bf).reshape(M // 128, 128, KT, 128)

    wqkv = prep_w(w_q, w_k, w_v)
    wo = prep_w(w_o)
    gb = np.empty((128, 2 * CB), np.float32)
    gb[:, 0:CB] = np.asarray(gamma, np.float32).reshape(CB, 128).T
    gb[:, CB:2 * CB] = np.asarray(beta, np.float32).reshape(CB, 128).T
    return xins, wqkv, wo, gb


def kernel(x, w_q, w_k, w_v, w_o, gamma, beta):
    from concourse.bass_utils import run_bass_kernel_spmd

    if "nc" not in _STATE:
        _STATE["nc"] = _build()
    nc = _STATE["nc"]

    x = np.asarray(x, np.float32)
    xins, wqkv, wo, gb = _prep_inputs(x, w_q, w_k, w_v, w_o, gamma, beta)
    in_maps = [{"xin": xins[c], "wqkv": wqkv, "wo": wo, "gb": gb}
               for c in range(N_CORES)]

    res = run_bass_kernel_spmd(nc, in_maps, core_ids=list(range(N_CORES)),
                               tmpdir=os.environ.get("HR2O_TMPDIR"))
    _STATE["last_results"] = res

    out = np.array(x, copy=True)
    for c in range(N_CORES):
        virt = res.results[c]["out"].astype(np.float32)   # [4, 512, 32, 32]
        out[4 * c:4 * c + 4] += virt
    return out
